# revision 5
# baseline (speedup 1.0000x reference)
"""Trainium2 Bass kernel for nn_ComplexAttention (B=8, C=512, H=W=32, HEADS=8).

Strategy
--------
Data-parallel over batch: one batch element per NeuronCore (8 cores), no
collectives.  Host-side algebraic fusion shrinks the per-core work:

  reference:  Q = R_q Wq Z,  K = R_k Wk Z,  V = R_v Wv Z   (complex, [C,T])
              S = Re(Q^H K)/sqrt(dh),  causal softmax -> A
              out = R_o Wo (V A^T)

  fused:      M = Wq^T diag(e^{i(phi_k-phi_q)}) Wk / sqrt(dh)   (host, f64)
              N = diag(e^{i phi_o}) Wo diag(e^{i phi_v}) Wv     (host, f64)
              Y = M Z            (channel-major [C,T])
              S = Re(Z^H Y)      = Zre^T Yre + Zim^T Yim
              A = softmax(causal(S))        (no max-subtraction: |S| < ~30)
              U = N Z            (token-major [T,C])
              out = U^T A^T      (channel-major [C,T], = re/im pair)

All PE work in bf16 (1 cyc/row at any N; halves LDWEIGHTS + DMA), PSUM
accumulates fp32.  End-to-end rel err ~8e-3 (budget 2e-2).

Schedule notes (from HW traces):
 - inputs stream on TWO engine queues (sync: mtre+zre; scalar: ntre+zim)
   so the per-queue ~200-300 GB/s DMA issue rate doesn't starve the PE.
 - softmax exp reads scores straight out of PSUM (scalar engine), row
   sums come free via accum_out; 1/l is folded into the PE "transpose"
   (plain matmul against diag(1/l) built on DVE from identity).
 - t-tiles 4..7 first, then out[:, 512:] is emitted (its DMA overlaps
   the scores of tiles 3..0), then out[:, :512].
 - outputs are bf16 (host casts back) and drain on two queues
   (sync: re, gpsimd: im); psum->sbuf copies are spread over
   vector/gpsimd/scalar so no single engine gates the tensor engine.
"""

import math

import numpy as np

import concourse.mybir as mybir
import concourse.tile as tile
from concourse import bacc
from concourse.bass_utils import run_bass_kernel_spmd

B, C, HH, WW = 8, 512, 32, 32
T = HH * WW          # 1024 tokens
DH = C // 8          # head dim (scale only)
P = 128
CT = C // P          # 4 channel tiles
TT = T // P          # 8 token tiles
NEG = -1.0e30

f32 = mybir.dt.float32
bf16 = mybir.dt.bfloat16
# kept for test.py compat
VALUE_BF16 = True
FULL_BF16 = True


def _mm(nc, out, lhsT, rhs, start, stop):
    nc.tensor.matmul(out, lhsT, rhs, start=start, stop=stop)


_CACHE: dict = {}


def _get_program(has_imag: bool):
    key = has_imag
    if key not in _CACHE:
        _CACHE[key] = _build_program(has_imag)
    return _CACHE[key]


def _build_program(has_imag: bool):
    nc = bacc.Bacc("TRN2", target_bir_lowering=False, debug=False)

    zre_d = nc.dram_tensor("zre", [C, T], bf16, kind="ExternalInput").ap()
    zim_d = nc.dram_tensor("zim", [C, T], bf16, kind="ExternalInput").ap()
    mtre_d = nc.dram_tensor("mtre", [C, C], bf16, kind="ExternalInput").ap()
    ntre_d = nc.dram_tensor("ntre", [C, C], bf16, kind="ExternalInput").ap()
    if has_imag:
        mtim_d = nc.dram_tensor("mtim", [C, C], bf16, kind="ExternalInput").ap()
        mtimn_d = nc.dram_tensor("mtimn", [C, C], bf16, kind="ExternalInput").ap()
        ntim_d = nc.dram_tensor("ntim", [C, C], bf16, kind="ExternalInput").ap()
        ntimn_d = nc.dram_tensor("ntimn", [C, C], bf16, kind="ExternalInput").ap()
    ident_d = nc.dram_tensor("ident", [P, P], bf16, kind="ExternalInput").ap()
    tri_d = nc.dram_tensor("tri", [P, P], f32, kind="ExternalInput").ap()
    outre_d = nc.dram_tensor("outre", [C, T], bf16, kind="ExternalOutput").ap()
    outim_d = nc.dram_tensor("outim", [C, T], bf16, kind="ExternalOutput").ap()

    with tile.TileContext(nc) as tc:
        with (
            tc.tile_pool(name="const", bufs=1) as cp,
            tc.tile_pool(name="work", bufs=4) as wp,
            tc.tile_pool(name="small", bufs=12) as sp,
            tc.tile_pool(name="psmm", bufs=6, space="PSUM") as pmm,
            tc.tile_pool(name="pstr", bufs=2, space="PSUM") as ptr,
        ):
            # -- persistent tiles ------------------------------------------
            ident = cp.tile([P, P], bf16, tag="ident", name="ident")
            tri = cp.tile([P, P], f32, tag="tri", name="tri")
            mtre = [cp.tile([P, C], bf16, tag=f"mtre{c}", name=f"mtre{c}")
                    for c in range(CT)]
            ntre = [cp.tile([P, C], bf16, tag=f"ntre{c}", name=f"ntre{c}")
                    for c in range(CT)]
            zre_h = [[cp.tile([P, 512], bf16, tag=f"zre{c}_{h}",
                              name=f"zre{c}_{h}") for c in range(CT)]
                     for h in range(2)]
            zim_h = [[cp.tile([P, 512], bf16, tag=f"zim{c}_{h}",
                              name=f"zim{c}_{h}") for c in range(CT)]
                     for h in range(2)]
            if has_imag:
                mtim = [cp.tile([P, C], bf16, tag=f"mtim{c}", name=f"mtim{c}")
                        for c in range(CT)]
                mtimn = [cp.tile([P, C], bf16, tag=f"mtimn{c}",
                                 name=f"mtimn{c}") for c in range(CT)]
                ntim = [cp.tile([P, C], bf16, tag=f"ntim{c}", name=f"ntim{c}")
                        for c in range(CT)]
                ntimn = [cp.tile([P, C], bf16, tag=f"ntimn{c}",
                                 name=f"ntimn{c}") for c in range(CT)]
            yre = [[cp.tile([P, 512], bf16, tag=f"yre{c}_{n}",
                            name=f"yre{c}_{n}") for n in range(2)]
                   for c in range(CT)]
            yim = [[cp.tile([P, 512], bf16, tag=f"yim{c}_{n}",
                            name=f"yim{c}_{n}") for n in range(2)]
                   for c in range(CT)]
            ure = [cp.tile([P, C], bf16, tag=f"ure{j}", name=f"ure{j}")
                   for j in range(TT)]
            uim = [cp.tile([P, C], bf16, tag=f"uim{j}", name=f"uim{j}")
                   for j in range(TT)]
            pt = {}
            for j in range(TT):
                for n in range(2):
                    if n == 0 and j >= 4:
                        continue
                    pt[(j, n)] = cp.tile([P, 512], bf16, tag=f"pt{j}_{n}",
                                         name=f"pt{j}_{n}")

            # -- input DMA: two queues, priority order ---------------------
            # qA feeds the first compute phase (Y_re), qB everything the
            # later phases need; both run concurrently on separate engines.
            qA, qB, qC = nc.sync, nc.scalar, nc.gpsimd
            qC.dma_start(out=ident, in_=ident_d)
            qC.dma_start(out=tri, in_=tri_d)
            for c in range(CT):
                qA.dma_start(out=mtre[c], in_=mtre_d[c * P:(c + 1) * P, :])
                qA.dma_start(out=zre_h[0][c],
                             in_=zre_d[c * P:(c + 1) * P, 0:512])
            for c in range(CT):
                qA.dma_start(out=zre_h[1][c],
                             in_=zre_d[c * P:(c + 1) * P, 512:1024])
            if has_imag:
                for c in range(CT):
                    qB.dma_start(out=mtimn[c],
                                 in_=mtimn_d[c * P:(c + 1) * P, :])
            for c in range(CT):
                qB.dma_start(out=zim_h[0][c],
                             in_=zim_d[c * P:(c + 1) * P, 0:512])
            for c in range(CT):
                qB.dma_start(out=zim_h[1][c],
                             in_=zim_d[c * P:(c + 1) * P, 512:1024])
            for c in range(CT):
                qB.dma_start(out=ntre[c], in_=ntre_d[c * P:(c + 1) * P, :])
            if has_imag:
                for c in range(CT):
                    qB.dma_start(out=mtim[c],
                                 in_=mtim_d[c * P:(c + 1) * P, :])
                for c in range(CT):
                    qB.dma_start(out=ntim[c],
                                 in_=ntim_d[c * P:(c + 1) * P, :])
                for c in range(CT):
                    qB.dma_start(out=ntimn[c],
                                 in_=ntimn_d[c * P:(c + 1) * P, :])

            copy_engines = [nc.vector, nc.scalar]

            def psum_to_sbuf(dst_ap, src_ap, k=0):
                eng = copy_engines[k % len(copy_engines)]
                if eng is nc.scalar:
                    eng.activation(out=dst_ap, in_=src_ap,
                                   func=mybir.ActivationFunctionType.Copy)
                else:
                    eng.tensor_copy(out=dst_ap, in_=src_ap)

            def emit_y(dst, terms):
                nterm = len(terms)
                for n in range(2):
                    pss = [pmm.tile([P, 512], f32, tag="mm", name="psmm")
                           for _ in range(CT)]
                    for t_i, (w, zh) in enumerate(terms):
                        for c in range(CT):
                            for m in range(CT):
                                _mm(nc, pss[m], w[c][:, m * P:(m + 1) * P],
                                    zh[n][c],
                                    start=(t_i == 0 and c == 0),
                                    stop=(t_i == nterm - 1 and c == CT - 1))
                    for m in range(CT):
                        psum_to_sbuf(dst[m][n], pss[m], k=m)

            def emit_u(dst, terms):
                for j in range(TT):
                    usl = slice((j % 4) * P, (j % 4 + 1) * P)
                    ps = pmm.tile([P, 512], f32, tag="mm", name="psmm")
                    nacc = len(terms) * CT
                    k = 0
                    for zh, w in terms:
                        for c in range(CT):
                            _mm(nc, ps, zh[j // 4][c][:, usl], w[c][:, :],
                                start=(k == 0), stop=(k == nacc - 1))
                            k += 1
                    psum_to_sbuf(dst[j], ps, k=j)

            if not has_imag:
                emit_y(yre, [(mtre, zre_h)])
                emit_u(ure, [(zre_h, ntre)])
                emit_y(yim, [(mtre, zim_h)])
                emit_u(uim, [(zim_h, ntre)])
            else:
                emit_y(yre, [(mtre, zre_h), (mtimn, zim_h)])
                emit_y(yim, [(mtre, zim_h), (mtim, zre_h)])
                emit_u(ure, [(zre_h, ntre), (zim_h, ntimn)])
                emit_u(uim, [(zim_h, ntre), (zre_h, ntim)])

            def emit_out_chunk(n, half):
                """out[:, n*512:(n+1)*512] for re (half 0) / im (half 1)."""
                u, dram = ((ure, outre_d), (uim, outim_d))[half]
                oeng = (nc.sync, nc.gpsimd)[half]
                ceng = (nc.vector, nc.scalar)[half]
                tsl = slice(n * 512, (n + 1) * 512)
                js = list(range(min(4 * n + 4, 8) if n else 4))
                dview = dram.rearrange("(m p) t -> p m t", p=P)
                for mh in range(2):
                    o = wp.tile([P, 2, 512], bf16, tag="osb", name="osb")
                    for mi in range(2):
                        m = 2 * mh + mi
                        msl = slice(m * P, (m + 1) * P)
                        ps = pmm.tile([P, 512], f32, tag="mm", name="psmm")
                        for j in js:
                            lo = max(0, j * P - n * 512)
                            _mm(nc, ps[:, lo:512],
                                u[j][:, msl], pt[(j, n)][:, lo:512],
                                start=(j == js[0]), stop=(j == js[-1]))
                        if ceng is nc.scalar:
                            ceng.activation(
                                out=o[:, mi, :], in_=ps,
                                func=mybir.ActivationFunctionType.Copy)
                        else:
                            ceng.tensor_copy(out=o[:, mi, :], in_=ps)
                    oeng.dma_start(out=dview[:, 2 * mh:2 * mh + 2, tsl],
                                   in_=o)

            # -- scores / softmax / transposes per t-tile -------------------
            def emit_scores_tile(i):
                ui = (i + 1) * P
                isl = slice((i % 4) * P, (i % 4 + 1) * P)
                s_sb = wp.tile([P, T], bf16, tag="s", name="s_sb")
                nchunks = (ui + 511) // 512
                lparts = []
                for q in range(nchunks):
                    w = min(512, ui - q * 512)
                    ps = pmm.tile([P, 512], f32, tag="mm", name="psmm")
                    k = 0
                    for zh, y in ((zre_h, yre), (zim_h, yim)):
                        for c in range(CT):
                            _mm(nc, ps[:, :w], zh[i // 4][c][:, isl],
                                y[c][q][:, :w],
                                start=(k == 0), stop=(k == 2 * CT - 1))
                            k += 1
                    last = q == nchunks - 1
                    if last:
                        if w > P:
                            # non-frontier part: exp straight from PSUM
                            lp = sp.tile([P, 1], f32, tag="lp", name="lp")
                            nc.scalar.activation(
                                out=s_sb[:, q * 512: q * 512 + w - P],
                                in_=ps[:, : w - P],
                                func=mybir.ActivationFunctionType.Exp,
                                accum_out=lp,
                            )
                            lparts.append(lp)
                        # frontier cols: +mask (DVE), then exp
                        fr = sp.tile([P, P], f32, tag="fr", name="fr")
                        nc.vector.tensor_add(out=fr,
                                             in0=ps[:, w - P: w],
                                             in1=tri)
                        lp = sp.tile([P, 1], f32, tag="lp", name="lp")
                        nc.scalar.activation(
                            out=s_sb[:, ui - P: ui],
                            in_=fr,
                            func=mybir.ActivationFunctionType.Exp,
                            accum_out=lp,
                        )
                        lparts.append(lp)
                    else:
                        lp = sp.tile([P, 1], f32, tag="lp", name="lp")
                        nc.scalar.activation(
                            out=s_sb[:, q * 512: q * 512 + w],
                            in_=ps[:, :w],
                            func=mybir.ActivationFunctionType.Exp,
                            accum_out=lp,
                        )
                        lparts.append(lp)

                lsum = lparts[0]
                for extra in lparts[1:]:
                    acc = sp.tile([P, 1], f32, tag="lacc", name="lacc")
                    nc.vector.tensor_add(out=acc, in0=lsum, in1=extra)
                    lsum = acc
                rl = sp.tile([P, 1], f32, tag="rl", name="rl")
                nc.vector.reciprocal(out=rl, in_=lsum)
                # diag(1/l) in bf16: the transpose matmul scales columns
                dg = sp.tile([P, P], bf16, tag="dg", name="dg")
                nc.vector.tensor_scalar_mul(dg, ident, rl)

                n = i // 4
                for j in range(i + 1):
                    pstile = ptr.tile([P, P], f32, tag="tr", name="pstile")
                    _mm(nc, pstile, s_sb[:, j * P:(j + 1) * P], dg,
                        start=True, stop=True)
                    nc.vector.tensor_copy(
                        out=pt[(j, n)][:, i * P - n * 512:
                                       (i + 1) * P - n * 512],
                        in_=pstile,
                    )

            for i in (4, 5, 6, 7):
                emit_scores_tile(i)
            emit_out_chunk(1, half=0)
            emit_scores_tile(3)
            emit_out_chunk(1, half=1)
            emit_scores_tile(2)
            emit_scores_tile(1)
            emit_scores_tile(0)
            emit_out_chunk(0, half=0)
            emit_out_chunk(0, half=1)

    nc.compile()
    return nc


def _prep_weights(Wq, phi_q, Wk, phi_k, Wv, phi_v, Wo, phi_o):
    Wq, Wk, Wv, Wo = (np.asarray(w, np.float64) for w in (Wq, Wk, Wv, Wo))
    pq, pk, pv, po = (np.asarray(p, np.float64)
                      for p in (phi_q, phi_k, phi_v, phi_o))
    M = (Wq.T @ (np.exp(1j * (pk - pq))[:, None] * Wk)) / math.sqrt(DH)
    N = (np.exp(1j * po)[:, None] * Wo) @ (np.exp(1j * pv)[:, None] * Wv)
    has_imag = not (np.allclose(M.imag, 0.0) and np.allclose(N.imag, 0.0))
    return M, N, has_imag


def kernel(z_re, z_im, Wq, phi_q, Wk, phi_k, Wv, phi_v, Wo, phi_o):
    import ml_dtypes
    snp = ml_dtypes.bfloat16
    z_re = np.ascontiguousarray(np.asarray(z_re, np.float32).astype(snp))
    z_im = np.ascontiguousarray(np.asarray(z_im, np.float32).astype(snp))
    M, N, has_imag = _prep_weights(Wq, phi_q, Wk, phi_k, Wv, phi_v, Wo, phi_o)

    consts = {
        "mtre": np.ascontiguousarray(M.real.T.astype(snp)),
        "ntre": np.ascontiguousarray(N.real.T.astype(snp)),
        "ident": np.eye(P, dtype=snp),
        "tri": np.triu(np.full((P, P), NEG, np.float32), 1),
    }
    if has_imag:
        mtim = np.ascontiguousarray(M.imag.T.astype(snp))
        ntim = np.ascontiguousarray(N.imag.T.astype(snp))
        consts.update(mtim=mtim, mtimn=-mtim, ntim=ntim, ntimn=-ntim)

    nc = _get_program(has_imag)
    in_maps = [
        dict(consts, zre=z_re[b].reshape(C, T), zim=z_im[b].reshape(C, T))
        for b in range(B)
    ]
    res = run_bass_kernel_spmd(nc, in_maps, list(range(B)))
    out_re = np.stack([np.asarray(res.results[b]["outre"], np.float32)
                       .reshape(C, HH, WW) for b in range(B)])
    out_im = np.stack([np.asarray(res.results[b]["outim"], np.float32)
                       .reshape(C, HH, WW) for b in range(B)])
    return out_re, out_im
